# revision 1
# baseline (speedup 1.0000x reference)
"""HashLayerFFN expert-parallel Trainium2 kernel.

Routing model: each token picks one of E=8 expert FFNs via a hash map.
Host side: group tokens by expert (cheap numpy), pad each expert bucket to
capacity C, and give expert i's weights + tokens to core i (expert-parallel,
one expert per core).  All matrices are pre-transposed on the host so the
device kernel is two dense back-to-back matmul phases with no on-chip
transposes:

  phase 1:  HidT[h, c] = relu( sum_d W1T[d, h] * XT[d, c] + b1[h] )
  phase 2:  YT[d, c]   =       sum_h W2T[h, d] * HidT[h, c] + b2[d]

The kernel is DMA-stream bound (16 MB of expert weights per core are used
exactly once), so the schedule is arranged to chase the weight stream:
 - W1 is DMA'd interleaved with XT so phase-1 matmuls start ~1us in;
 - phase 2 accumulates h-major across all 8 output PSUM banks concurrently,
   consuming each W2 h-tile as it lands; only the final h tile's 8 matmuls +
   the output stores trail the last weight byte.

PE matmuls default to fp16 operands (same 10-bit mantissa as TF32/float32r,
measured 4.1e-04 absmax relative error vs the fp32 reference) with fp32 PSUM
accumulation; fp16 halves the weight-stream bytes.  float32r ("f32r_f32r",
2.2e-04) is the fallback if tighter accuracy is ever needed.
"""

import numpy as np

B, S, D, H, E = 2, 1024, 1024, 2048, 8
N_CORES = 8
C = 320            # per-expert token capacity (seed-0 max bucket = 310;
                   # overflow tokens fall back to host numpy)
DT = 16            # h tiles of 128 in H
ND = 8             # d chunks of 128 in D
HG = 8             # h-tiles per W1 sbuf tile (DMA granularity)

# matmul dtype mode "layer1_layer2": each of f32r | f32 | fp16 | bf16
# (test.py may override before first call; shipped default = measured best)
MODE = "fp16_fp16"

# extra kwargs for run_bass_kernel_spmd (test.py sets trace=True here);
# LAST_RES holds the most recent BassKernelResults for profiling.
RUN_KWARGS = {}
LAST_RES = None

_cache = {}


def _np_dt(name):
    if name == "bf16":
        import ml_dtypes
        return np.dtype(ml_dtypes.bfloat16)
    if name == "fp16":
        return np.dtype(np.float16)
    return np.dtype(np.float32)


def _build_nc(mode):
    import concourse.mybir as mybir
    from concourse import bacc
    from concourse.tile import TileContext

    f32 = mybir.dt.float32
    mmdt = {
        "f32r": mybir.dt.float32r,
        "f32": f32,
        "fp16": mybir.dt.float16,
        "bf16": mybir.dt.bfloat16,
    }
    l1, l2 = mode.split("_")
    dt1, dt2 = mmdt[l1], mmdt[l2]

    nc = bacc.Bacc(None, target_bir_lowering=False)
    xt = nc.dram_tensor("xt", [128, ND, C], dt1, kind="ExternalInput")
    w1t = nc.dram_tensor("w1t", [ND, 128, DT, 128], dt1, kind="ExternalInput")
    bt = nc.dram_tensor("bt", [128, DT + ND], f32, kind="ExternalInput")
    w2t = nc.dram_tensor("w2t", [DT, 128, D], dt2, kind="ExternalInput")
    yt = nc.dram_tensor("yt", [ND, 128, C], f32, kind="ExternalOutput")

    with TileContext(nc) as tc:
        with (
            tc.tile_pool(name="consts", bufs=1) as consts,
            tc.tile_pool(name="xpool", bufs=1) as xpool,
            tc.tile_pool(name="w1pool", bufs=1) as w1pool,
            tc.tile_pool(name="w2pool", bufs=1) as w2pool,
            tc.tile_pool(name="hpool", bufs=1) as hpool,
            tc.tile_pool(name="ypool", bufs=4) as ypool,
            tc.tile_pool(name="ps1p", bufs=4, space="PSUM") as ps1p,
            tc.tile_pool(name="ps2p", bufs=4, space="PSUM") as ps2p,
        ):
            # stream order: biases, XT (one DMA), W1 tiles (d-major inside
            # each h-group so phase-1 consumes them as they land), W2 tiles
            xtile = xpool.tile([128, ND, C], dt1, name="xtile")
            nc.scalar.dma_start(out=xtile[:, 0:2, :], in_=xt[:, 0:2, :])
            nc.scalar.dma_start(out=xtile[:, 2:ND, :], in_=xt[:, 2:ND, :])
            xts = [xtile[:, d, :] for d in range(ND)]
            bts = consts.tile([128, DT + ND], f32)
            nc.scalar.dma_start(out=bts, in_=bt[:])
            b1s, b2s = bts[:, 0:DT], bts[:, DT:DT + ND]
            w1s = [[None] * (DT // HG) for _ in range(ND)]
            for g in range(DT // HG):
                for d in range(ND):
                    w1tile = w1pool.tile([128, HG, 128], dt1, name=f"w1_{d}_{g}")
                    nc.sync.dma_start(
                        out=w1tile, in_=w1t[d, :, g * HG:(g + 1) * HG, :]
                    )
                    w1s[d][g] = w1tile

            # W2 tiles stream after W1 (phase-2 consumes them h-major)
            w2s = []
            for h in range(DT):
                w2tile = w2pool.tile([128, D], dt2, name=f"w2_{h}")
                nc.sync.dma_start(out=w2tile, in_=w2t[h])
                w2s.append(w2tile)

            # phase 1: HidT tiles [128(hf), C], one per h tile.  The first
            # two h groups run d-interleaved so each arriving W1 tile feeds
            # two matmuls immediately (PE otherwise starves on the first
            # group); the rest run h-major (stream stays ahead by then).
            hids = [None] * DT
            NI = 6
            # 4 slots from ps1p + 2 borrowed from ps2p (idle until phase 2;
            # released by the h=4/5 relus long before phase-2 d=0..3 opens)
            ps1_01 = [ps1p.tile([128, C], f32, name="ps1") for _ in range(4)] +                      [ps2p.tile([128, C], f32, name="ps2") for _ in range(2)]
            for d in range(ND):
                for h in range(NI):
                    nc.tensor.matmul(
                        ps1_01[h],
                        lhsT=w1s[d][0][:, h, :],
                        rhs=xts[d],
                        start=(d == 0),
                        stop=(d == ND - 1),
                    )
            for h in range(NI):
                hid = hpool.tile([128, C], dt2, name=f"hid{h}")
                nc.scalar.activation(
                    out=hid, in_=ps1_01[h],
                    func=mybir.ActivationFunctionType.Relu,
                    bias=b1s[:, h:h + 1],
                )
                hids[h] = hid
            for h in range(NI, DT):
                ps1 = ps1p.tile([128, C], f32, name="ps1")
                g, j = h // HG, h % HG
                for d in range(ND):
                    nc.tensor.matmul(
                        ps1,
                        lhsT=w1s[d][g][:, j, :],
                        rhs=xts[d],
                        start=(d == 0),
                        stop=(d == ND - 1),
                    )
                hid = hpool.tile([128, C], dt2, name=f"hid{h}")
                nc.scalar.activation(
                    out=hid, in_=ps1,
                    func=mybir.ActivationFunctionType.Relu,
                    bias=b1s[:, h:h + 1],
                )
                hids[h] = hid

            # phase 2: 8 concurrently-open PSUM accumulation groups (6 banks
            # from ps2p + 2 reusing ps1p slots), h-major so each W2 tile is
            # consumed as it arrives off the wire.
            pgs = []
            for d in range(ND):
                if d < 4:
                    pg = ps2p.tile([128, C], f32, name="ps2")
                else:
                    pg = ps1p.tile([128, C], f32, name="ps1")
                pgs.append(pg)

            def bias_store(d, ysb):
                if d % 2 == 0:
                    nc.scalar.activation(
                        out=ysb, in_=pgs[d],
                        func=mybir.ActivationFunctionType.Identity,
                        bias=b2s[:, d:d + 1],
                    )
                else:
                    nc.vector.tensor_scalar_add(ysb, pgs[d], b2s[:, d:d + 1])
                nc.sync.dma_start(out=yt[d], in_=ysb)

            if mybir.dt.size(dt2) == 2:
                # fast W2 stream: h-major only while chasing the wire, then
                # group-major so groups close staggered and stores overlap
                # the remaining matmuls
                H1 = DT // 2
                for h in range(H1):
                    for d in range(ND):
                        nc.tensor.matmul(
                            pgs[d],
                            lhsT=w2s[h][:, d * 128:(d + 1) * 128],
                            rhs=hids[h],
                            start=(h == 0),
                            stop=False,
                        )
                for d in range(ND):
                    for h in range(H1, DT):
                        nc.tensor.matmul(
                            pgs[d],
                            lhsT=w2s[h][:, d * 128:(d + 1) * 128],
                            rhs=hids[h],
                            start=False,
                            stop=(h == DT - 1),
                        )
                    ysb = ypool.tile([128, C], f32, name="ysb")
                    bias_store(d, ysb)
            else:
                for h in range(DT):
                    for d in range(ND):
                        nc.tensor.matmul(
                            pgs[d],
                            lhsT=w2s[h][:, d * 128:(d + 1) * 128],
                            rhs=hids[h],
                            start=(h == 0),
                            stop=(h == DT - 1),
                        )
                for d in range(ND):
                    ysb = ypool.tile([128, C], f32, name="ysb")
                    bias_store(d, ysb)

    nc.finalize()
    return nc


def _get_nc():
    if MODE not in _cache:
        _cache[MODE] = _build_nc(MODE)
    return _cache[MODE]


def kernel(x, orig_input, hash_map, W1, b1, W2, b2, **_unused):
    from concourse import bass_utils

    x = np.asarray(x)
    W1 = np.asarray(W1, dtype=np.float32)
    b1 = np.asarray(b1, dtype=np.float32)
    W2 = np.asarray(W2, dtype=np.float32)
    b2 = np.asarray(b2, dtype=np.float32)
    l1, l2 = MODE.split("_")
    dt1, dt2 = _np_dt(l1), _np_dt(l2)

    xf = np.ascontiguousarray(x, dtype=np.float32).reshape(B * S, D)
    e = np.asarray(hash_map).astype(np.int64)[
        np.asarray(orig_input).astype(np.int64).reshape(-1)
    ]
    order = np.argsort(e, kind="stable")
    counts = np.bincount(e, minlength=E)
    starts = np.zeros(E + 1, dtype=np.int64)
    starts[1:] = np.cumsum(counts)

    in_maps = []
    overflow = []          # (expert, token idx array) done on host (rare)
    idxs = []
    for i in range(E):
        idx = order[starts[i]:starts[i + 1]]
        if len(idx) > C:
            overflow.append((i, idx[C:]))
            idx = idx[:C]
        idxs.append(idx)
        xe = np.zeros((C, D), dtype=np.float32)
        xe[: len(idx)] = xf[idx]
        in_maps.append({
            "xt": np.ascontiguousarray(
                xe.T.reshape(ND, 128, C).transpose(1, 0, 2)).astype(dt1),
            "w1t": np.ascontiguousarray(W1[i].T).astype(dt1)
                     .reshape(ND, 128, DT, 128),
            "w2t": np.ascontiguousarray(W2[i].T).astype(dt2)
                     .reshape(DT, 128, D),
            "bt": np.ascontiguousarray(np.concatenate(
                [b1[i].reshape(DT, 128).T, b2[i].reshape(ND, 128).T], axis=1)),
        })

    nc = _get_nc()
    res = bass_utils.run_bass_kernel_spmd(
        nc, in_maps, core_ids=list(range(N_CORES)), **RUN_KWARGS
    )
    global LAST_RES
    LAST_RES = res

    out = np.zeros((B * S, D), dtype=np.float32)
    for i in range(E):
        idx = idxs[i]
        y = res.results[i]["yt"].reshape(D, C).T  # [C, D]
        out[idx] = y[: len(idx)]
    for i, idx in overflow:   # host fallback for bucket overflow (rare)
        hh = np.maximum(xf[idx] @ W1[i].T + b1[i], 0.0)
        out[idx] = hh @ W2[i].T + b2[i]
    return out.reshape(B, S, D)



# revision 17
# speedup vs baseline: 1.1448x; 1.1448x over previous
"""HashLayerFFN Trainium2 kernel — H-split expert pairs.

Experts are paired (largest bucket with smallest); each pair maps to two
cores.  A core holds HALF the hidden dim (1024 rows) of BOTH experts in its
pair and processes ALL their tokens; the two cores' partial y are summed on
the host (b2 added there too).  Per-core PE work is balanced at
~2*(nA+nB)/2 tokens-equivalent instead of the max bucket.

Phases per core: warmup, 1A (x_A @ W1A_half), 2A (partial yA), 1B, 2B.
One packed DMA stream: A-segs (x_A-d | W1A-d), W2A h-chunks, B-segs, W2B.
"""

import numpy as np

B, S, D, H, E = 2, 1024, 1024, 2048, 8
N_CORES = 8
NP = 4             # expert pairs
CA = 310           # capacity for the big expert of each pair
CB = 252           # capacity for the small expert
DT8 = 8            # h-tiles per half (H/2 = 1024)
ND = 8             # d chunks of 128 in D

N_WARM = 32
WARM_COLS = 64

SEG_A = CA + 1024
SEG_B = CB + 1024
OFF_W2A = ND * SEG_A
OFF_B = OFF_W2A + DT8 * 1024
OFF_W2B = OFF_B + ND * SEG_B
TOT = OFF_W2B + DT8 * 1024

MODE = "fp16_fp16"   # kept for test.py compatibility
RUN_KWARGS = {}
LAST_RES = None
_cache = {}


def _build_nc():
    import concourse.mybir as mybir
    from concourse import bacc
    from concourse.tile import TileContext

    f32 = mybir.dt.float32
    f16 = mybir.dt.float16
    dt1 = f16

    nc = bacc.Bacc(None, target_bir_lowering=False)
    st = nc.dram_tensor("st", [128, TOT], dt1, kind="ExternalInput")
    bt = nc.dram_tensor("bt", [128, 2 * DT8], f32, kind="ExternalInput")
    yta = nc.dram_tensor("yta", [128, ND, CA], f16, kind="ExternalOutput")
    ytb = nc.dram_tensor("ytb", [128, ND, CB], f16, kind="ExternalOutput")

    with TileContext(nc) as tc:
        with (
            tc.tile_pool(name="consts", bufs=1) as consts,
            tc.tile_pool(name="spool", bufs=1) as spool,
            tc.tile_pool(name="hpool", bufs=1) as hpool,
            tc.tile_pool(name="yapool", bufs=1) as yapool,
            tc.tile_pool(name="ybpool", bufs=1) as ybpool,
            tc.tile_pool(name="warm", bufs=1) as warmp,
            tc.tile_pool(name="ps1p", bufs=4, space="PSUM") as ps1p,
            tc.tile_pool(name="ps2p", bufs=4, space="PSUM") as ps2p,
        ):
            wtile = warmp.tile([128, 128], dt1, name="wtile")
            nc.vector.memset(wtile, 0)

            stile = spool.tile([128, TOT], dt1, name="stile")
            for d in range(ND):
                nc.sync.dma_start(
                    out=stile[:, d * SEG_A:(d + 1) * SEG_A],
                    in_=st[:, d * SEG_A:(d + 1) * SEG_A],
                )
            bts = consts.tile([128, 2 * DT8], f32)
            nc.scalar.dma_start(out=bts, in_=bt[:])
            b1a, b1b = bts[:, 0:DT8], bts[:, DT8:2 * DT8]
            for j in range(DT8):
                nc.sync.dma_start(
                    out=stile[:, OFF_W2A + j * 1024:OFF_W2A + (j + 1) * 1024],
                    in_=st[:, OFF_W2A + j * 1024:OFF_W2A + (j + 1) * 1024],
                )
            for d in range(ND):
                nc.sync.dma_start(
                    out=stile[:, OFF_B + d * SEG_B:OFF_B + (d + 1) * SEG_B],
                    in_=st[:, OFF_B + d * SEG_B:OFF_B + (d + 1) * SEG_B],
                )
            for j in range(DT8):
                nc.sync.dma_start(
                    out=stile[:, OFF_W2B + j * 1024:OFF_W2B + (j + 1) * 1024],
                    in_=st[:, OFF_W2B + j * 1024:OFF_W2B + (j + 1) * 1024],
                )

            xa = [stile[:, d * SEG_A:d * SEG_A + CA] for d in range(ND)]
            xb = [stile[:, OFF_B + d * SEG_B:OFF_B + d * SEG_B + CB]
                  for d in range(ND)]

            def w1a(d, j):
                off = d * SEG_A + CA + j * 128
                return stile[:, off:off + 128]

            def w1b(d, j):
                off = OFF_B + d * SEG_B + CB + j * 128
                return stile[:, off:off + 128]

            def w2a(j, d):
                off = OFF_W2A + j * 1024 + d * 128
                return stile[:, off:off + 128]

            def w2b(j, d):
                off = OFF_W2B + j * 1024 + d * 128
                return stile[:, off:off + 128]

            # ---- phase 1A: 8 h-tiles d-interleaved, chasing A-segs
            psa = [ps1p.tile([128, CA], f32, name="ps1") for _ in range(4)] + \
                  [ps2p.tile([128, CA], f32, name="ps2") for _ in range(4)]
            for i in range(N_WARM):
                pw = psa[1 + i % 7]
                nc.tensor.matmul(
                    pw[:, :WARM_COLS], lhsT=wtile, rhs=wtile[:, :WARM_COLS],
                    start=True, stop=True,
                )
            for d in range(ND):
                for j in range(DT8):
                    nc.tensor.matmul(
                        psa[j], lhsT=w1a(d, j), rhs=xa[d],
                        start=(d == 0), stop=(d == ND - 1),
                    )
            hida = []
            for j in range(DT8):
                hid = hpool.tile([128, CA], dt1, name=f"hida{j}")
                nc.scalar.activation(
                    out=hid, in_=psa[j],
                    func=mybir.ActivationFunctionType.Relu,
                    bias=b1a[:, j:j + 1],
                )
                hida.append(hid)

            # ---- phase 2A: 8 d-groups, j-major while chasing W2A, then
            # d-major so groups close staggered
            pga = [ps2p.tile([128, CA], f32, name="ps2") for _ in range(4)] + \
                  [ps1p.tile([128, CA], f32, name="ps1") for _ in range(4)]
            J1 = DT8 // 2
            for j in range(J1):
                for d in range(ND):
                    nc.tensor.matmul(
                        pga[d], lhsT=w2a(j, d), rhs=hida[j],
                        start=(j == 0), stop=False,
                    )
            ya = yapool.tile([128, ND, CA], f16, name="ya")
            for d in range(ND):
                for j in range(J1, DT8):
                    nc.tensor.matmul(
                        pga[d], lhsT=w2a(j, d), rhs=hida[j],
                        start=False, stop=(j == DT8 - 1),
                    )
                if d % 2 == 0:
                    nc.scalar.activation(
                        out=ya[:, d, :], in_=pga[d],
                        func=mybir.ActivationFunctionType.Identity,
                    )
                else:
                    nc.vector.tensor_copy(ya[:, d, :], pga[d])
                if d == 3:
                    nc.sync.dma_start(out=yta[:, 0:4, :], in_=ya[:, 0:4, :])
                elif d == ND - 1:
                    nc.sync.dma_start(out=yta[:, 4:8, :], in_=ya[:, 4:8, :])

            # ---- phase 1B
            psb = [ps1p.tile([128, CB], f32, name="ps1") for _ in range(4)] + \
                  [ps2p.tile([128, CB], f32, name="ps2") for _ in range(4)]
            for d in range(ND):
                for j in range(DT8):
                    nc.tensor.matmul(
                        psb[j], lhsT=w1b(d, j), rhs=xb[d],
                        start=(d == 0), stop=(d == ND - 1),
                    )
            hidb = []
            for j in range(DT8):
                hid = hpool.tile([128, CB], dt1, name=f"hidb{j}")
                nc.scalar.activation(
                    out=hid, in_=psb[j],
                    func=mybir.ActivationFunctionType.Relu,
                    bias=b1b[:, j:j + 1],
                )
                hidb.append(hid)

            # ---- phase 2B
            pgb = [ps2p.tile([128, CB], f32, name="ps2") for _ in range(4)] + \
                  [ps1p.tile([128, CB], f32, name="ps1") for _ in range(4)]
            for j in range(J1):
                for d in range(ND):
                    nc.tensor.matmul(
                        pgb[d], lhsT=w2b(j, d), rhs=hidb[j],
                        start=(j == 0), stop=False,
                    )
            yb = ybpool.tile([128, ND, CB], f16, name="yb")
            for d in range(ND):
                for j in range(J1, DT8):
                    nc.tensor.matmul(
                        pgb[d], lhsT=w2b(j, d), rhs=hidb[j],
                        start=False, stop=(j == DT8 - 1),
                    )
                if d % 2 == 0:
                    nc.scalar.activation(
                        out=yb[:, d, :], in_=pgb[d],
                        func=mybir.ActivationFunctionType.Identity,
                    )
                else:
                    nc.vector.tensor_copy(yb[:, d, :], pgb[d])
                if d == 3:
                    nc.sync.dma_start(out=ytb[:, 0:4, :], in_=yb[:, 0:4, :])
                elif d == 6:
                    nc.scalar.dma_start(out=ytb[:, 4:7, :], in_=yb[:, 4:7, :])
                elif d == ND - 1:
                    nc.sync.dma_start(out=ytb[:, 7:8, :], in_=yb[:, 7:8, :])

    nc.finalize()
    return nc


def get_nc():
    if "b" not in _cache:
        _cache["b"] = _build_nc()
    return _cache["b"]


_get_nc = get_nc   # test.py compatibility


def kernel(x, orig_input, hash_map, W1, b1, W2, b2, **_unused):
    from concourse import bass_utils

    x = np.asarray(x)
    W1 = np.asarray(W1, dtype=np.float32)
    b1 = np.asarray(b1, dtype=np.float32)
    W2 = np.asarray(W2, dtype=np.float32)
    b2 = np.asarray(b2, dtype=np.float32)

    xf = np.ascontiguousarray(x, dtype=np.float32).reshape(B * S, D)
    e = np.asarray(hash_map).astype(np.int64)[
        np.asarray(orig_input).astype(np.int64).reshape(-1)
    ]
    order = np.argsort(e, kind="stable")
    counts = np.bincount(e, minlength=E)
    starts = np.zeros(E + 1, dtype=np.int64)
    starts[1:] = np.cumsum(counts)

    # pair largest with smallest
    rank = np.argsort(-counts, kind="stable")
    pairs = [(int(rank[p]), int(rank[E - 1 - p])) for p in range(NP)]

    overflow = []
    tok = {}
    for i in range(E):
        cap = CA if any(p[0] == i for p in pairs) else CB
        idx = order[starts[i]:starts[i + 1]]
        if len(idx) > cap:
            overflow.append((i, idx[cap:]))
            idx = idx[:cap]
        tok[i] = idx

    def xpack(idx, cap):
        xe = np.zeros((cap, D), dtype=np.float32)
        xe[: len(idx)] = xf[idx]
        return xe.T.reshape(ND, 128, cap)       # [d, 128, cap]

    in_maps = []
    for p, (a, b) in enumerate(pairs):
        xta = xpack(tok[a], CA).astype(np.float16)
        xtb = xpack(tok[b], CB).astype(np.float16)
        for hh in range(2):
            r0 = hh * 1024
            w1ah = W1[a][r0:r0 + 1024].T.reshape(ND, 128, DT8, 128)
            w1bh = W1[b][r0:r0 + 1024].T.reshape(ND, 128, DT8, 128)
            w2ah = W2[a][:, r0:r0 + 1024].T.reshape(DT8, 128, D)
            w2bh = W2[b][:, r0:r0 + 1024].T.reshape(DT8, 128, D)
            stream = np.empty((128, TOT), dtype=np.float16)
            for d in range(ND):
                seg = stream[:, d * SEG_A:(d + 1) * SEG_A]
                seg[:, :CA] = xta[d]
                seg[:, CA:] = w1ah[d].reshape(128, 1024)
                seg = stream[:, OFF_B + d * SEG_B:OFF_B + (d + 1) * SEG_B]
                seg[:, :CB] = xtb[d]
                seg[:, CB:] = w1bh[d].reshape(128, 1024)
            stream[:, OFF_W2A:OFF_W2A + DT8 * 1024] = \
                w2ah.transpose(1, 0, 2).reshape(128, DT8 * 1024)
            stream[:, OFF_W2B:] = \
                w2bh.transpose(1, 0, 2).reshape(128, DT8 * 1024)
            bta = b1[a][r0:r0 + 1024].reshape(DT8, 128).T
            btb = b1[b][r0:r0 + 1024].reshape(DT8, 128).T
            in_maps.append({
                "st": stream,
                "bt": np.ascontiguousarray(
                    np.concatenate([bta, btb], axis=1)),
            })

    nc = get_nc()
    res = bass_utils.run_bass_kernel_spmd(
        nc, in_maps, core_ids=list(range(N_CORES)), **RUN_KWARGS
    )
    global LAST_RES
    LAST_RES = res

    out = np.zeros((B * S, D), dtype=np.float32)
    for p, (a, b) in enumerate(pairs):
        r0, r1 = res.results[2 * p], res.results[2 * p + 1]
        ya = (r0["yta"].astype(np.float32) + r1["yta"].astype(np.float32))
        yb = (r0["ytb"].astype(np.float32) + r1["ytb"].astype(np.float32))
        ya = ya.transpose(1, 0, 2).reshape(D, CA).T + b2[a]   # [CA, D]
        yb = yb.transpose(1, 0, 2).reshape(D, CB).T + b2[b]
        out[tok[a]] = ya[: len(tok[a])]
        out[tok[b]] = yb[: len(tok[b])]
    for i, idx in overflow:
        hh = np.maximum(xf[idx] @ W1[i].T + b1[i], 0.0)
        out[idx] = hh @ W2[i].T + b2[i]
    return out.reshape(B, S, D)


# revision 18
# speedup vs baseline: 1.1458x; 1.0008x over previous
"""HashLayerFFN Trainium2 kernel — H-split expert pairs.

Experts are paired (largest bucket with smallest); each pair maps to two
cores.  A core holds HALF the hidden dim (1024 rows) of BOTH experts in its
pair and processes ALL their tokens; the two cores' partial y are summed on
the host (b2 added there too).  Per-core PE work is balanced at
~2*(nA+nB)/2 tokens-equivalent instead of the max bucket.

Phases per core: warmup, 1A (x_A @ W1A_half), 2A (partial yA), 1B, 2B.
One packed DMA stream: A-segs (x_A-d | W1A-d), W2A h-chunks, B-segs, W2B.
"""

import numpy as np

B, S, D, H, E = 2, 1024, 1024, 2048, 8
N_CORES = 8
NP = 4             # expert pairs
CA = 310           # capacity for the big expert of each pair
CB = 252           # capacity for the small expert
DT8 = 8            # h-tiles per half (H/2 = 1024)
ND = 8             # d chunks of 128 in D

N_WARM = 100
WARM_COLS = 32

SEG_A = CA + 1024
SEG_B = CB + 1024
OFF_W2A = ND * SEG_A
OFF_B = OFF_W2A + DT8 * 1024
OFF_W2B = OFF_B + ND * SEG_B
TOT = OFF_W2B + DT8 * 1024

MODE = "fp16_fp16"   # kept for test.py compatibility
RUN_KWARGS = {}
LAST_RES = None
_cache = {}


def _build_nc():
    import concourse.mybir as mybir
    from concourse import bacc
    from concourse.tile import TileContext

    f32 = mybir.dt.float32
    f16 = mybir.dt.float16
    dt1 = f16

    nc = bacc.Bacc(None, target_bir_lowering=False)
    st = nc.dram_tensor("st", [128, TOT], dt1, kind="ExternalInput")
    bt = nc.dram_tensor("bt", [128, 2 * DT8], f32, kind="ExternalInput")
    yta = nc.dram_tensor("yta", [128, ND, CA], f16, kind="ExternalOutput")
    ytb = nc.dram_tensor("ytb", [128, ND, CB], f16, kind="ExternalOutput")

    with TileContext(nc) as tc:
        with (
            tc.tile_pool(name="consts", bufs=1) as consts,
            tc.tile_pool(name="spool", bufs=1) as spool,
            tc.tile_pool(name="hpool", bufs=1) as hpool,
            tc.tile_pool(name="yapool", bufs=1) as yapool,
            tc.tile_pool(name="ybpool", bufs=1) as ybpool,
            tc.tile_pool(name="warm", bufs=1) as warmp,
            tc.tile_pool(name="ps1p", bufs=4, space="PSUM") as ps1p,
            tc.tile_pool(name="ps2p", bufs=4, space="PSUM") as ps2p,
        ):
            wtile = warmp.tile([128, 128], dt1, name="wtile")
            nc.vector.memset(wtile, 0)

            stile = spool.tile([128, TOT], dt1, name="stile")
            for d in range(ND):
                nc.sync.dma_start(
                    out=stile[:, d * SEG_A:(d + 1) * SEG_A],
                    in_=st[:, d * SEG_A:(d + 1) * SEG_A],
                )
            bts = consts.tile([128, 2 * DT8], f32)
            nc.scalar.dma_start(out=bts, in_=bt[:])
            b1a, b1b = bts[:, 0:DT8], bts[:, DT8:2 * DT8]
            for j in range(DT8):
                nc.sync.dma_start(
                    out=stile[:, OFF_W2A + j * 1024:OFF_W2A + (j + 1) * 1024],
                    in_=st[:, OFF_W2A + j * 1024:OFF_W2A + (j + 1) * 1024],
                )
            for d in range(ND):
                nc.sync.dma_start(
                    out=stile[:, OFF_B + d * SEG_B:OFF_B + (d + 1) * SEG_B],
                    in_=st[:, OFF_B + d * SEG_B:OFF_B + (d + 1) * SEG_B],
                )
            for j in range(DT8):
                nc.sync.dma_start(
                    out=stile[:, OFF_W2B + j * 1024:OFF_W2B + (j + 1) * 1024],
                    in_=st[:, OFF_W2B + j * 1024:OFF_W2B + (j + 1) * 1024],
                )

            xa = [stile[:, d * SEG_A:d * SEG_A + CA] for d in range(ND)]
            xb = [stile[:, OFF_B + d * SEG_B:OFF_B + d * SEG_B + CB]
                  for d in range(ND)]

            def w1a(d, j):
                off = d * SEG_A + CA + j * 128
                return stile[:, off:off + 128]

            def w1b(d, j):
                off = OFF_B + d * SEG_B + CB + j * 128
                return stile[:, off:off + 128]

            def w2a(j, d):
                off = OFF_W2A + j * 1024 + d * 128
                return stile[:, off:off + 128]

            def w2b(j, d):
                off = OFF_W2B + j * 1024 + d * 128
                return stile[:, off:off + 128]

            # ---- phase 1A: 8 h-tiles d-interleaved, chasing A-segs
            psa = [ps1p.tile([128, CA], f32, name="ps1") for _ in range(4)] + \
                  [ps2p.tile([128, CA], f32, name="ps2") for _ in range(4)]
            for i in range(N_WARM):
                pw = psa[1 + i % 7]
                nc.tensor.matmul(
                    pw[:, :WARM_COLS], lhsT=wtile, rhs=wtile[:, :WARM_COLS],
                    start=True, stop=True,
                )
            for d in range(ND):
                for j in range(DT8):
                    nc.tensor.matmul(
                        psa[j], lhsT=w1a(d, j), rhs=xa[d],
                        start=(d == 0), stop=(d == ND - 1),
                    )
            hida = []
            for j in range(DT8):
                hid = hpool.tile([128, CA], dt1, name=f"hida{j}")
                nc.scalar.activation(
                    out=hid, in_=psa[j],
                    func=mybir.ActivationFunctionType.Relu,
                    bias=b1a[:, j:j + 1],
                )
                hida.append(hid)

            # ---- phase 2A: 8 d-groups, j-major while chasing W2A, then
            # d-major so groups close staggered
            pga = [ps2p.tile([128, CA], f32, name="ps2") for _ in range(4)] + \
                  [ps1p.tile([128, CA], f32, name="ps1") for _ in range(4)]
            J1 = DT8 // 2
            for j in range(J1):
                for d in range(ND):
                    nc.tensor.matmul(
                        pga[d], lhsT=w2a(j, d), rhs=hida[j],
                        start=(j == 0), stop=False,
                    )
            ya = yapool.tile([128, ND, CA], f16, name="ya")
            for d in range(ND):
                for j in range(J1, DT8):
                    nc.tensor.matmul(
                        pga[d], lhsT=w2a(j, d), rhs=hida[j],
                        start=False, stop=(j == DT8 - 1),
                    )
                if d % 2 == 0:
                    nc.scalar.activation(
                        out=ya[:, d, :], in_=pga[d],
                        func=mybir.ActivationFunctionType.Identity,
                    )
                else:
                    nc.vector.tensor_copy(ya[:, d, :], pga[d])
                if d == 3:
                    nc.sync.dma_start(out=yta[:, 0:4, :], in_=ya[:, 0:4, :])
                elif d == ND - 1:
                    nc.sync.dma_start(out=yta[:, 4:8, :], in_=ya[:, 4:8, :])

            # ---- phase 1B
            psb = [ps1p.tile([128, CB], f32, name="ps1") for _ in range(4)] + \
                  [ps2p.tile([128, CB], f32, name="ps2") for _ in range(4)]
            for d in range(ND):
                for j in range(DT8):
                    nc.tensor.matmul(
                        psb[j], lhsT=w1b(d, j), rhs=xb[d],
                        start=(d == 0), stop=(d == ND - 1),
                    )
            hidb = []
            for j in range(DT8):
                hid = hpool.tile([128, CB], dt1, name=f"hidb{j}")
                nc.scalar.activation(
                    out=hid, in_=psb[j],
                    func=mybir.ActivationFunctionType.Relu,
                    bias=b1b[:, j:j + 1],
                )
                hidb.append(hid)

            # ---- phase 2B
            pgb = [ps2p.tile([128, CB], f32, name="ps2") for _ in range(4)] + \
                  [ps1p.tile([128, CB], f32, name="ps1") for _ in range(4)]
            for j in range(J1):
                for d in range(ND):
                    nc.tensor.matmul(
                        pgb[d], lhsT=w2b(j, d), rhs=hidb[j],
                        start=(j == 0), stop=False,
                    )
            yb = ybpool.tile([128, ND, CB], f16, name="yb")
            for d in range(ND):
                for j in range(J1, DT8):
                    nc.tensor.matmul(
                        pgb[d], lhsT=w2b(j, d), rhs=hidb[j],
                        start=False, stop=(j == DT8 - 1),
                    )
                if d % 2 == 0:
                    nc.scalar.activation(
                        out=yb[:, d, :], in_=pgb[d],
                        func=mybir.ActivationFunctionType.Identity,
                    )
                else:
                    nc.vector.tensor_copy(yb[:, d, :], pgb[d])
                if d == 3:
                    nc.sync.dma_start(out=ytb[:, 0:4, :], in_=yb[:, 0:4, :])
                elif d == 6:
                    nc.scalar.dma_start(out=ytb[:, 4:7, :], in_=yb[:, 4:7, :])
                elif d == ND - 1:
                    nc.sync.dma_start(out=ytb[:, 7:8, :], in_=yb[:, 7:8, :])

    nc.finalize()
    return nc


def get_nc():
    if "b" not in _cache:
        _cache["b"] = _build_nc()
    return _cache["b"]


_get_nc = get_nc   # test.py compatibility


def kernel(x, orig_input, hash_map, W1, b1, W2, b2, **_unused):
    from concourse import bass_utils

    x = np.asarray(x)
    W1 = np.asarray(W1, dtype=np.float32)
    b1 = np.asarray(b1, dtype=np.float32)
    W2 = np.asarray(W2, dtype=np.float32)
    b2 = np.asarray(b2, dtype=np.float32)

    xf = np.ascontiguousarray(x, dtype=np.float32).reshape(B * S, D)
    e = np.asarray(hash_map).astype(np.int64)[
        np.asarray(orig_input).astype(np.int64).reshape(-1)
    ]
    order = np.argsort(e, kind="stable")
    counts = np.bincount(e, minlength=E)
    starts = np.zeros(E + 1, dtype=np.int64)
    starts[1:] = np.cumsum(counts)

    # pair largest with smallest
    rank = np.argsort(-counts, kind="stable")
    pairs = [(int(rank[p]), int(rank[E - 1 - p])) for p in range(NP)]

    overflow = []
    tok = {}
    for i in range(E):
        cap = CA if any(p[0] == i for p in pairs) else CB
        idx = order[starts[i]:starts[i + 1]]
        if len(idx) > cap:
            overflow.append((i, idx[cap:]))
            idx = idx[:cap]
        tok[i] = idx

    def xpack(idx, cap):
        xe = np.zeros((cap, D), dtype=np.float32)
        xe[: len(idx)] = xf[idx]
        return xe.T.reshape(ND, 128, cap)       # [d, 128, cap]

    in_maps = []
    for p, (a, b) in enumerate(pairs):
        xta = xpack(tok[a], CA).astype(np.float16)
        xtb = xpack(tok[b], CB).astype(np.float16)
        for hh in range(2):
            r0 = hh * 1024
            w1ah = W1[a][r0:r0 + 1024].T.reshape(ND, 128, DT8, 128)
            w1bh = W1[b][r0:r0 + 1024].T.reshape(ND, 128, DT8, 128)
            w2ah = W2[a][:, r0:r0 + 1024].T.reshape(DT8, 128, D)
            w2bh = W2[b][:, r0:r0 + 1024].T.reshape(DT8, 128, D)
            stream = np.empty((128, TOT), dtype=np.float16)
            for d in range(ND):
                seg = stream[:, d * SEG_A:(d + 1) * SEG_A]
                seg[:, :CA] = xta[d]
                seg[:, CA:] = w1ah[d].reshape(128, 1024)
                seg = stream[:, OFF_B + d * SEG_B:OFF_B + (d + 1) * SEG_B]
                seg[:, :CB] = xtb[d]
                seg[:, CB:] = w1bh[d].reshape(128, 1024)
            stream[:, OFF_W2A:OFF_W2A + DT8 * 1024] = \
                w2ah.transpose(1, 0, 2).reshape(128, DT8 * 1024)
            stream[:, OFF_W2B:] = \
                w2bh.transpose(1, 0, 2).reshape(128, DT8 * 1024)
            bta = b1[a][r0:r0 + 1024].reshape(DT8, 128).T
            btb = b1[b][r0:r0 + 1024].reshape(DT8, 128).T
            in_maps.append({
                "st": stream,
                "bt": np.ascontiguousarray(
                    np.concatenate([bta, btb], axis=1)),
            })

    nc = get_nc()
    res = bass_utils.run_bass_kernel_spmd(
        nc, in_maps, core_ids=list(range(N_CORES)), **RUN_KWARGS
    )
    global LAST_RES
    LAST_RES = res

    out = np.zeros((B * S, D), dtype=np.float32)
    for p, (a, b) in enumerate(pairs):
        r0, r1 = res.results[2 * p], res.results[2 * p + 1]
        ya = (r0["yta"].astype(np.float32) + r1["yta"].astype(np.float32))
        yb = (r0["ytb"].astype(np.float32) + r1["ytb"].astype(np.float32))
        ya = ya.transpose(1, 0, 2).reshape(D, CA).T + b2[a]   # [CA, D]
        yb = yb.transpose(1, 0, 2).reshape(D, CB).T + b2[b]
        out[tok[a]] = ya[: len(tok[a])]
        out[tok[b]] = yb[: len(tok[b])]
    for i, idx in overflow:
        hh = np.maximum(xf[idx] @ W1[i].T + b1[i], 0.0)
        out[idx] = hh @ W2[i].T + b2[i]
    return out.reshape(B, S, D)
